# revision 13
# baseline (speedup 1.0000x reference)
"""GCN layer (dense projection + sparse neighbor aggregation) on 8 Trainium2
NeuronCores via Bass/Tile.

Strategy: shard nodes (and their incident edges, grouped by destination row)
across the 8 cores; replicate W/b; AllGather the projected node features so
every core can gather arbitrary source columns; per 128-row output block,
bulk-gather the needed source rows with DMAGatherAnt (int16 indices into 4
sub-tables of <=32k rows), scale by edge_val, and segment-sum via an
assignment-matrix matmul accumulated in PSUM (bias folded in as an extra
rank-128 matmul; padded gather slots are killed by rowloc=-1).
"""

import sys

if "/opt/trn_rl_repo" not in sys.path:
    sys.path.insert(0, "/opt/trn_rl_repo")

import numpy as np

import concourse.bass as bass
import concourse.mybir as mybir
import concourse.tile as tile
from concourse import bacc
from concourse.bass_utils import run_bass_kernel_spmd

N_NODES = 100000
N_EDGES = 1600000
IN_FT = 256
OUT_FT = 64
NCORES = 8
NS = N_NODES // NCORES          # 12500 nodes per core
NB = (NS + 127) // 128          # 98 row blocks per core
NSP = NB * 128                  # 12544 padded nodes per core
VROWS = NCORES * NSP            # 100352 rows in the gathered feature table
NSUB = 4                        # sub-tables (int16 index range)
SUBROWS = VROWS // NSUB         # 25088 rows per sub-table
GB = 7                          # row blocks per pipeline group (98 = 14 * 7)
NGROUPS = NB // GB

F32 = mybir.dt.float32
F16 = mybir.dt.float16
I32 = mybir.dt.int32
I16 = mybir.dt.int16


def build_program(nchb: int):
    """One SPMD Bass program; all 8 cores run it on their own shards.

    nchb: 128-edge chunks per (row-block, sub-table bucket).
    """
    ncht = NSUB * nchb          # chunks per row block
    sg = GB * nchb              # chunks per (group, bucket) stream
    NQ = 4                      # SWDGE queues for parallel descriptor gen
    MAXCH = 8                   # 1024 indices = HW cap per dma_gather
    nc = bacc.Bacc("TRN2", target_bir_lowering=False, debug=False,
                   num_devices=NCORES, num_swdge_queues=NQ)

    seqT = nc.dram_tensor("seqT", [2, 128, NSP], F32, kind="ExternalInput")
    gidx = nc.dram_tensor("gidx", [128, NGROUPS, NSUB, sg * 8], I16,
                          kind="ExternalInput")
    vr = nc.dram_tensor("vr", [128, NB, ncht], I32, kind="ExternalInput")
    w_in = nc.dram_tensor("w", [128, 2, OUT_FT], F32, kind="ExternalInput")
    bias_in = nc.dram_tensor("biasb", [128, OUT_FT], F16,
                             kind="ExternalInput")
    # partition-major layouts: [p, block, feature]; host un-permutes
    sf_out = nc.dram_tensor("sf", [128, NB, OUT_FT], F32,
                            kind="ExternalOutput")
    agg_out = nc.dram_tensor("agg", [128, NB, OUT_FT], F32,
                             kind="ExternalOutput")
    ccin = nc.dram_tensor("ccin", [128, NB, OUT_FT], F32)
    xfull = nc.dram_tensor("xfull", [VROWS, OUT_FT], F32, addr_space="Shared")

    groups = [list(range(NCORES))]

    with tile.TileContext(nc) as tc:
        with (
            tc.tile_pool(name="const", bufs=1) as cpool,
            tc.tile_pool(name="psum", bufs=2, space="PSUM") as psum_pool,
        ):
            w_sb = cpool.tile([128, 2, OUT_FT], F32)
            nc.sync.dma_start(out=w_sb[:], in_=w_in[:])
            # bias/128 broadcast, fp16: added into PSUM via a ones matmul
            bias_sb = cpool.tile([128, OUT_FT], F16)
            nc.sync.dma_start(out=bias_sb[:], in_=bias_in[:])
            ones_sb = cpool.tile([128, 128], F16)
            nc.gpsimd.memset(ones_sb[:], 1.0)
            iota_i = cpool.tile([128, 128], I32)
            nc.gpsimd.iota(iota_i[:], pattern=[[1, 128]], base=0,
                           channel_multiplier=0)
            iota_f = cpool.tile([128, 128], F16)
            nc.vector.tensor_copy(out=iota_f[:], in_=iota_i[:])

            # ---- phase 1: x = seq @ W (fp32), write sf + ccin ----
            with (
                tc.tile_pool(name="seqpool", bufs=1) as seqpool,
                tc.tile_pool(name="p1work", bufs=3) as p1work,
            ):
                seqT_sb = seqpool.tile([128, 2, NSP], F32)
                nc.sync.dma_start(out=seqT_sb[:, 0, :], in_=seqT[0])
                nc.sync.dma_start(out=seqT_sb[:, 1, :], in_=seqT[1])

                for g in range(NGROUPS):
                    x_sb = p1work.tile([128, GB, OUT_FT], F32, tag="x_sb")
                    for j in range(GB):
                        nb = g * GB + j
                        px = psum_pool.tile([128, OUT_FT], F32, tag="px")
                        for kc in range(2):
                            nc.tensor.matmul(
                                px[:],
                                seqT_sb[:, kc, nb * 128:(nb + 1) * 128],
                                w_sb[:, kc, :],
                                start=(kc == 0),
                                stop=(kc == 1),
                            )
                        nc.vector.tensor_copy(out=x_sb[:, j, :], in_=px[:])
                    nc.sync.dma_start(
                        out=sf_out[:, g * GB:(g + 1) * GB, :], in_=x_sb[:])
                    nc.sync.dma_start(
                        out=ccin[:, g * GB:(g + 1) * GB, :], in_=x_sb[:])

            # ---- halo exchange: AllGather f32 features ----
            nc.gpsimd.collective_compute(
                "AllGather",
                mybir.AluOpType.bypass,
                replica_groups=groups,
                ins=[ccin[:]],
                outs=[xfull[:]],
            )

            # ---- phase 2: bulk gather + scale + segment-sum matmul ----
            gather_q = [0]
            with tc.tile_pool(name="p2work", bufs=3) as p2:
                for g in range(NGROUPS):
                    gidx_sb = p2.tile([128, NSUB, sg * 8], I16, tag="gidx")
                    nc.sync.dma_start(out=gidx_sb[:], in_=gidx[:, g])
                    vr_sb = p2.tile([128, GB, ncht], I32, tag="vr")
                    nc.sync.dma_start(out=vr_sb[:],
                                      in_=vr[:, g * GB:(g + 1) * GB, :])
                    # xg[p, b, j*nchb+cc, :] = xfull rows for bucket b,
                    # block j, chunk cc
                    xg = p2.tile([128, NSUB, sg, OUT_FT], F32, tag="xg")
                    for b in range(NSUB):
                        for off in range(0, sg, MAXCH):
                            ln = min(MAXCH, sg - off)
                            nc.gpsimd.dma_gather(
                                out_ap=xg[:, b, off:off + ln, :],
                                in_ap=xfull[b * SUBROWS:(b + 1) * SUBROWS, :],
                                idxs_ap=gidx_sb[:, b, off * 8:(off + ln) * 8],
                                num_idxs=ln * 128,
                                num_idxs_reg=ln * 128,
                                elem_size=OUT_FT,
                                queue_num=gather_q[0] % NQ,
                            )
                            gather_q[0] += 1
                    vr16 = vr_sb[:].bitcast(F16)  # [128, GB, 2*ncht]
                    out_sb = p2.tile([128, GB, OUT_FT], F32, tag="out_sb")
                    for j in range(GB):
                        vj = vr16[:, j, :].rearrange("p (c two) -> p c two",
                                                     two=2)
                        val = vj[:, :, 0:1]          # [128, ncht, 1]
                        rloc = vj[:, :, 1:2]
                        # fold edge_val in while casting f32 -> fp16
                        xg16 = p2.tile([128, ncht * OUT_FT], F16, tag="xg16")
                        xg_j = xg[:, :, j * nchb:(j + 1) * nchb, :]
                        nc.vector.tensor_tensor(
                            out=xg16[:].rearrange("p (b c f) -> p b c f",
                                                  b=NSUB, c=nchb),
                            in0=xg_j,
                            in1=val.rearrange("p (b c) one -> p b (c one)",
                                              b=NSUB).unsqueeze(3)
                            .broadcast_to([128, NSUB, nchb, OUT_FT]),
                            op=mybir.AluOpType.mult,
                        )
                        # A[p, c, q] = (rowloc[p, c] == q); -1 pads vanish
                        a_sb = p2.tile([128, ncht * 128], F16, tag="a_sb")
                        nc.vector.tensor_tensor(
                            out=a_sb[:].rearrange("p (c q) -> p c q", q=128),
                            in0=rloc.broadcast_to([128, ncht, 128]),
                            in1=iota_f[:].unsqueeze(1).broadcast_to(
                                [128, ncht, 128]),
                            op=mybir.AluOpType.is_equal,
                        )
                        po = psum_pool.tile([128, OUT_FT], F32, tag="po")
                        nc.tensor.matmul(po[:], ones_sb[:], bias_sb[:],
                                         start=True, stop=False)
                        for ci in range(ncht):
                            nc.tensor.matmul(
                                po[:],
                                a_sb[:, ci * 128:(ci + 1) * 128],
                                xg16[:, ci * OUT_FT:(ci + 1) * OUT_FT],
                                start=False,
                                stop=(ci == ncht - 1),
                            )
                        nc.scalar.activation(
                            out=out_sb[:, j, :], in_=po[:],
                            func=mybir.ActivationFunctionType.Relu)
                    nc.sync.dma_start(
                        out=agg_out[:, g * GB:(g + 1) * GB, :], in_=out_sb[:])

    nc.compile()
    return nc


def prepare_inputs(seq, edge_row, edge_col, edge_val, W, b):
    """Host-side sharding / graph partitioning. Returns (in_maps, nchb)."""
    seq = np.asarray(seq, dtype=np.float32).reshape(N_NODES, IN_FT)
    r = np.asarray(edge_row).astype(np.int64)
    c = np.asarray(edge_col).astype(np.int64)
    v = np.asarray(edge_val, dtype=np.float32)
    W = np.asarray(W, dtype=np.float32).reshape(IN_FT, OUT_FT)
    b = np.asarray(b, dtype=np.float32).reshape(OUT_FT)

    # feature-table row (partition-major shard layout) and sub-table bucket
    csrc = c // NS
    crem = c % NS
    colmap = csrc * NSP + (crem % 128) * NB + crem // 128
    bucket = colmap // SUBROWS
    lidx = (colmap % SUBROWS).astype(np.int16)

    core = r // NS
    loc = r - core * NS
    blk = loc >> 7
    rowloc = (loc & 127).astype(np.float16)

    # sort edges by (core, block, bucket)
    key = (core * NB + blk) * NSUB + bucket
    order = np.argsort(key, kind="stable")
    key_s = key[order]
    lidx_s = lidx[order]
    val_s = v[order].astype(np.float16)
    rloc_s = rowloc[order]

    ngrp = NCORES * NB * NSUB
    starts = np.searchsorted(key_s, np.arange(ngrp))
    deg = np.diff(np.append(starts, N_EDGES))
    nchb = max(1, int(np.ceil(deg.max() / 128)))
    cap = nchb * 128

    pos = np.arange(N_EDGES) - starts[key_s]
    dest = key_s * cap + pos
    idxp = np.zeros(ngrp * cap, np.int16)          # pad: row 0 (valid)
    valp = np.zeros(ngrp * cap, np.float16)
    rlp = np.full(ngrp * cap, -1.0, np.float16)    # pad: killed in A
    idxp[dest] = lidx_s
    valp[dest] = val_s
    rlp[dest] = rloc_s

    # gather indices, wrapped: edge i of a (group, bucket) stream sits at
    # [i % 16, i // 16], replicated to all 8 16-partition groups
    idxp = idxp.reshape(NCORES, NGROUPS, GB, NSUB, cap)
    # (group,bucket) stream concatenates the GB blocks
    idxp = idxp.transpose(0, 1, 3, 2, 4).reshape(
        NCORES, NGROUPS, NSUB, GB * cap)
    wrapped = idxp.reshape(NCORES, NGROUPS, NSUB, GB * cap // 16, 16)
    wrapped = wrapped.transpose(0, 4, 1, 2, 3)  # [cores, 16, ngrp, nsub, S]
    gidx = np.broadcast_to(wrapped[:, None],
                           (NCORES, 8, 16, NGROUPS, NSUB, GB * cap // 16))
    gidx = np.ascontiguousarray(
        gidx.reshape(NCORES, 128, NGROUPS, NSUB, GB * cap // 16))

    # val/rowloc packed per block: [p, block, ncht] with chunk index
    # c = bucket * nchb + cc, lane p = edge % 128
    packed = (valp.view(np.uint16).astype(np.uint32)
              | (rlp.view(np.uint16).astype(np.uint32) << 16)).view(np.int32)
    packed = packed.reshape(NCORES, NB, NSUB, nchb, 128)
    vr = np.ascontiguousarray(packed.transpose(0, 4, 1, 2, 3).reshape(
        NCORES, 128, NB, NSUB * nchb))

    biasb = np.broadcast_to((b / 128.0).astype(np.float16),
                            (128, OUT_FT)).copy()
    w3 = np.ascontiguousarray(
        W.reshape(2, 128, OUT_FT).transpose(1, 0, 2))  # [128, 2, OUT_FT]

    in_maps = []
    for k in range(NCORES):
        shard = np.zeros((NSP, IN_FT), np.float32)
        shard[:NS] = seq[k * NS:(k + 1) * NS]
        seqT_k = np.ascontiguousarray(shard.T).reshape(2, 128, NSP)
        in_maps.append({
            "seqT": seqT_k,
            "gidx": gidx[k],
            "vr": vr[k],
            "w": w3,
            "biasb": biasb,
        })
    return in_maps, nchb


_PROGRAMS: dict[int, object] = {}


def kernel(seq, edge_row, edge_col, edge_val, W, b):
    in_maps, nchb = prepare_inputs(seq, edge_row, edge_col, edge_val, W, b)
    prog = _PROGRAMS.get(nchb)
    if prog is None:
        prog = _PROGRAMS[nchb] = build_program(nchb)
    res = run_bass_kernel_spmd(prog, in_maps, core_ids=list(range(NCORES)))

    def unshard(name):
        # [128, NB, OUT_FT] partition-major -> [NS, OUT_FT] row-major
        parts = [
            res.results[k][name].transpose(1, 0, 2).reshape(NSP, OUT_FT)[:NS]
            for k in range(NCORES)
        ]
        return np.concatenate(parts)[None]

    return unshard("agg"), unshard("sf")
